# revision 8
# baseline (speedup 1.0000x reference)
"""Trainium2 Bass kernel v3 for the DEFT Bishop-frame rod problem.

Design (driven by CoreSim + device-resident timing: device time is
DVE-element-bound):
- fp16 compute tiles: 2x_1P DVE mode on tensor_tensor, 4x on copies, and
  small enough that the whole pipeline stays SBUF-resident (no DRAM scratch,
  no input reload).
- u0's first cross (init_direct x e0) rides in column 0 of the full-width kb
  cross pass (e5 column 0 holds init_direct).
- Scan: 7-level Hillis-Steele quaternion scan; per level 3 permute-build
  copies (2 DVE + negations on ACT where strides allow), 4 in-place [W,4,n]
  mults, 3 adds.  State q lives in q8[0:4]; A1 scratch in q8[4:8]; A2/A3 in A8.
- Apply reuses q8[4:6] for qv dup planes.
- Output staged in four double-buffered fp16 [W,32,15] chunks; SWDGE cast-DMA writes fp32 DRAM.
"""
import sys

sys.path.insert(0, "/opt/trn_rl_repo")

import numpy as np
import concourse.bass as bass
import concourse.mybir as mybir
from concourse import tile
from concourse.bass_utils import run_bass_kernel_spmd

AF = mybir.ActivationFunctionType
ALU = mybir.AluOpType
F32 = mybir.dt.float32
F16 = mybir.dt.float16

NCORES = 8
NV = 129
E = 128
P = 128
ERR = 1e-6
MAG_THR = float(np.float32(4.0 * (1.0 - (1.0 - ERR) ** 2) / (1.0 - ERR) ** 2))

_CACHE = {}


def build_nc(R, reps=1):
    W = R // P
    assert R % P == 0
    nc = bass.Bass()
    v = nc.vector
    sc = nc.scalar

    verts = nc.dram_tensor("verts", [R, NV, 3], F32, kind="ExternalInput")
    init_d = nc.dram_tensor("init_direct", [R, 3], F32, kind="ExternalInput")
    m_theta = nc.dram_tensor("m_theta", [R, E], F32, kind="ExternalInput")
    restL = nc.dram_tensor("restEdgeL", [R, E], F32, kind="ExternalInput")
    out = nc.dram_tensor("out", [R, E, 5, 3], F32, kind="ExternalOutput")

    vr = verts[:].rearrange("(p w) n c -> p w n c", p=P)
    ir = init_d[:].rearrange("(p w) c -> p w c", p=P)
    tr = m_theta[:].rearrange("(p w) e -> p w e", p=P)
    lr = restL[:].rearrange("(p w) e -> p w e", p=P)
    outr = out[:].rearrange("(p w) e f c -> p w e f c", p=P)

    with tile.TileContext(nc) as tc, nc.allow_low_precision(
            reason="fp16 pipeline; rel-err gate is 2e-2, measured ~7e-4"):
     for _rep in range(reps):
      with tc.tile_pool(name="outer", bufs=1) as outer:
        c4 = outer.tile([P, 1], F32)
        v.memset(c4[:], 4.0)
        chpi = outer.tile([P, 1], F32, tag="chpi")
        v.memset(chpi[:], float(np.pi / 2))
        c0 = outer.tile([P, 1], F32, tag="c0")
        v.memset(c0[:], 0.0)

        # e5 columns: 0 = init_direct, 1..128 = edges; planes x,y,z,x,y
        e5 = outer.tile([P, W, 5, NV], F16, tag="e5")
        kb = outer.tile([P, W, 3, E], F16, tag="kb")   # col 0: -N0 (not output)
        u05 = outer.tile([P, W, 5], F32, tag="u05")
        bu = outer.tile([P, W, 5, E], F16, tag="bu")
        cosf = outer.tile([P, W, E], F16, tag="cosf")
        sinf = outer.tile([P, W, E], F16, tag="sinf")

        # ---------- edges + init_direct column (verts DMA issues first) --
        with tc.tile_pool(name="cv", bufs=1) as cv:
            vf = cv.tile([P, W, NV, 3], F32)
            wq = W // 4
            for qi in range(4):
                nc.sync.dma_start(vf[:, qi * wq : (qi + 1) * wq],
                                  vr[:, qi * wq : (qi + 1) * wq])
            d3 = cv.tile([P, W, 3], F32, tag="d3")
            nc.sync.dma_start(d3[:], ir[:, :, :])
            # trig rides in this pool: th DMA after verts, Sin on idle ACT
            th = cv.tile([P, W, E], F32, tag="th")
            nc.sync.dma_start(th[:], tr[:])
            for qi in range(4):
                wl, whi = qi * wq, (qi + 1) * wq
                v.tensor_tensor(
                    out=e5[:, wl:whi, 0:3, 1:NV],
                    in0=vf[:, wl:whi, 1:NV, :].rearrange("p w n c -> p w c n"),
                    in1=vf[:, wl:whi, 0 : NV - 1, :].rearrange("p w n c -> p w c n"),
                    op=ALU.subtract)
            v.tensor_copy(out=e5[:, :, 0:3, 0], in_=d3[:])
            v.tensor_copy(out=e5[:, :, 3:5, :], in_=e5[:, :, 0:2, :])
            sc.activation(cosf[:], th[:], AF.Sin, bias=chpi[:])
            sc.activation(sinf[:], th[:], AF.Sin, bias=c0[:])

        with tc.tile_pool(name="qp", bufs=1) as qp:
            q8 = qp.tile([P, W, 8, E], F16)   # state q in 0:4, scratch 4:8
            u0b = qp.tile([P, W, 5, 64], F16, tag="u0b")  # u0 bcast (col-invariant)

            # ---------- kb cross pass (col 0 -> -N0), quaternions --------
            with tc.tile_pool(name="cw", bufs=1) as cw:
                Lf = cw.tile([P, W, E], F16)
                nc.gpsimd.dma_start(Lf[:], lr[:])
                tp = bu[:, :, 0:3, :]          # bu reused as scratch until apply
                dd = cw.tile([P, W, E], F16, tag="dd")
                den = cw.tile([P, W, E], F16, tag="den")
                g = cw.tile([P, W, E], F16, tag="g")
                ep = lambda i: e5[:, :, i : i + 3, 0:E]
                en = lambda i: e5[:, :, i : i + 3, 1 : E + 1]
                # kb <- e_prev x e_next  (col j: cross(col_j, col_j+1))
                v.tensor_tensor(out=kb[:], in0=ep(1), in1=en(2), op=ALU.mult)
                v.tensor_tensor(out=tp, in0=ep(2), in1=en(1), op=ALU.mult)
                v.tensor_tensor(out=kb[:], in0=kb[:], in1=tp, op=ALU.subtract)
                # dd = e_prev . e_next
                v.tensor_tensor(out=tp, in0=ep(0), in1=en(0), op=ALU.mult)
                v.tensor_tensor(out=dd[:], in0=bu[:, :, 0, :], in1=bu[:, :, 1, :],
                                op=ALU.add)
                v.tensor_tensor(out=dd[:], in0=dd[:], in1=bu[:, :, 2, :], op=ALU.add)

                # ---------- u0 (uses kb col 0 = -N0, before kb scaled) ---
                n5 = cw.tile([P, W, 5], F32, tag="n5")
                t3 = cw.tile([P, W, 3], F32, tag="t3")
                s3 = cw.tile([P, W, 3], F32, tag="s3")
                nn = cw.tile([P, W], F32, tag="nn")
                v.tensor_copy(out=n5[:, :, 0:3], in_=kb[:, :, :, 0])
                v.tensor_copy(out=n5[:, :, 3:5], in_=n5[:, :, 0:2])
                e05 = e5[:, :, :, 1]          # first edge (dup planes)
                v.tensor_tensor(out=t3[:], in0=n5[:, :, 1:4], in1=e05[:, :, 2:5],
                                op=ALU.mult)
                v.tensor_tensor(out=s3[:], in0=n5[:, :, 2:5], in1=e05[:, :, 1:4],
                                op=ALU.mult)
                v.tensor_tensor(out=t3[:], in0=t3[:], in1=s3[:], op=ALU.subtract)
                v.tensor_tensor(out=s3[:], in0=t3[:], in1=t3[:], op=ALU.mult)
                v.tensor_reduce(out=nn[:], in_=s3[:], axis=mybir.AxisListType.X,
                                op=ALU.add)
                sc.activation(nn[:], nn[:], AF.Sqrt, bias=c0[:])
                v.reciprocal(out=nn[:], in_=nn[:])
                v.tensor_scalar_mul(nn[:], nn[:], -1.0)   # sign: t3 = -u0_unnorm
                nb = nn[:].unsqueeze(2).to_broadcast([P, W, 3])
                v.tensor_tensor(out=u05[:, :, 0:3], in0=t3[:], in1=nb, op=ALU.mult)
                v.tensor_copy(out=u05[:, :, 3:5], in_=u05[:, :, 0:2])
                sc.activation(u0b[:],
                              u05[:].unsqueeze(3).to_broadcast([P, W, 5, 64]),
                              AF.Copy)

                # ---------- kb scale + quaternions (cols 1..127) ---------
                v.tensor_tensor(out=den[:, :, 1:E], in0=Lf[:, :, 0 : E - 1],
                                in1=Lf[:, :, 1:E], op=ALU.mult)
                v.tensor_tensor(out=den[:, :, 1:E], in0=den[:, :, 1:E],
                                in1=dd[:, :, 1:E], op=ALU.add)
                v.tensor_scalar_mul(den[:, :, 1:E], den[:, :, 1:E], 0.5)
                v.reciprocal(out=den[:, :, 1:E], in_=den[:, :, 1:E])
                dnb = den[:, :, 1:E].unsqueeze(2).to_broadcast([P, W, 3, E - 1])
                v.tensor_tensor(out=kb[:, :, :, 1:E], in0=kb[:, :, :, 1:E],
                                in1=dnb, op=ALU.mult)
                # mag = |kb|^2
                v.tensor_tensor(out=bu[:, :, 0:3, 1:E], in0=kb[:, :, :, 1:E],
                                in1=kb[:, :, :, 1:E], op=ALU.mult)
                v.tensor_tensor(out=dd[:, :, 1:E], in0=bu[:, :, 0, 1:E],
                                in1=bu[:, :, 1, 1:E], op=ALU.add)
                v.tensor_tensor(out=dd[:, :, 1:E], in0=dd[:, :, 1:E],
                                in1=bu[:, :, 2, 1:E], op=ALU.add)
                # rsq = 1/sqrt(4+mag); g = mag > thr; fg = rsq*g
                sc.activation(den[:, :, 1:E], dd[:, :, 1:E], AF.Sqrt, bias=c4[:])
                v.reciprocal(out=den[:, :, 1:E], in_=den[:, :, 1:E])
                v.tensor_scalar(g[:, :, 1:E], dd[:, :, 1:E], MAG_THR, None,
                                op0=ALU.is_gt)
                v.tensor_tensor(out=den[:, :, 1:E], in0=den[:, :, 1:E],
                                in1=g[:, :, 1:E], op=ALU.mult)
                fgb = den[:, :, 1:E].unsqueeze(2).to_broadcast([P, W, 3, E - 1])
                v.tensor_tensor(out=q8[:, :, 1:4, 1:E], in0=kb[:, :, :, 1:E],
                                in1=fgb, op=ALU.mult)
                # w = 2*fg + (1 - g)
                v.tensor_scalar(dd[:, :, 1:E], den[:, :, 1:E], 2.0, 1.0,
                                op0=ALU.mult, op1=ALU.add)
                v.scalar_tensor_tensor(out=q8[:, :, 0, 1:E], in0=g[:, :, 1:E],
                                       scalar=-1.0, in1=dd[:, :, 1:E],
                                       op0=ALU.mult, op1=ALU.add)
                v.memset(q8[:, :, 0, 0:1], 1.0)
                v.memset(q8[:, :, 1:4, 0:1], 0.0)

            # ---------- scan: 7 doubling levels --------------------------
            with tc.tile_pool(name="sp", bufs=1) as sp:
                A8 = sp.tile([P, W, 8, E], F16)
                for k in range(7):
                    h = 1 << k
                    n = E - h
                    blo = lambda kc: (q8[:, :, kc, 0:n].unsqueeze(2)
                                      .to_broadcast([P, W, 4, n]))
                    # A1 = (-x, w, z, -y) into q8[4:8]
                    v.tensor_copy(out=q8[:, :, 5:7, h:E], in_=q8[:, :, 0:4:3, h:E])
                    sc.activation(q8[:, :, 4:8:3, h:E], q8[:, :, 1:3, h:E],
                                  AF.Copy, scale=-1.0)
                    # A2 = (-y, -z, w, x) into A8[0:4]
                    sc.activation(A8[:, :, 2:4, h:E], q8[:, :, 0:2, h:E], AF.Copy)
                    sc.activation(A8[:, :, 0:2, h:E], q8[:, :, 2:4, h:E],
                                  AF.Copy, scale=-1.0)
                    # A3 = (-z, y, -x, w) into A8[4:8]
                    v.tensor_copy(out=A8[:, :, 5:8:2, h:E], in_=q8[:, :, 2::-2, h:E])
                    v.tensor_scalar_mul(A8[:, :, 4:7:2, h:E], q8[:, :, 3::-2, h:E],
                                        -1.0)
                    # products (in place) + accumulate
                    v.tensor_tensor(out=q8[:, :, 4:8, h:E], in0=q8[:, :, 4:8, h:E],
                                    in1=blo(1), op=ALU.mult)
                    v.tensor_tensor(out=A8[:, :, 0:4, h:E], in0=A8[:, :, 0:4, h:E],
                                    in1=blo(2), op=ALU.mult)
                    v.tensor_tensor(out=A8[:, :, 4:8, h:E], in0=A8[:, :, 4:8, h:E],
                                    in1=blo(3), op=ALU.mult)
                    v.tensor_tensor(out=q8[:, :, 4:8, h:E], in0=q8[:, :, 4:8, h:E],
                                    in1=A8[:, :, 0:4, h:E], op=ALU.add)
                    v.tensor_tensor(out=A8[:, :, 0:4, h:E], in0=q8[:, :, 0:4, h:E],
                                    in1=blo(0), op=ALU.mult)
                    v.tensor_tensor(out=q8[:, :, 4:8, h:E], in0=q8[:, :, 4:8, h:E],
                                    in1=A8[:, :, 4:8, h:E], op=ALU.add)
                    v.tensor_tensor(out=q8[:, :, 0:4, h:E], in0=q8[:, :, 4:8, h:E],
                                    in1=A8[:, :, 0:4, h:E], op=ALU.add)

            # ---------- apply: b_u = rot(Q, u0) --------------------------
            with tc.tile_pool(name="ap", bufs=1) as ap:
                # qv dup planes into q8[4:6] -> qv windows q8[2:5], q8[3:6]
                v.tensor_copy(out=q8[:, :, 4:6, :], in_=q8[:, :, 1:3, :])
                for hf in range(2):
                    s = slice(hf * 64, hf * 64 + 64)
                    uv5 = ap.tile([P, W, 5, 64], F16, tag="uv5", name="uv5")
                    t1 = ap.tile([P, W, 3, 64], F16, tag="t1", name="t1")
                    t2 = ap.tile([P, W, 3, 64], F16, tag="t2", name="t2")
                    # uv = qv x u0
                    v.tensor_tensor(out=t1[:], in0=q8[:, :, 2:5, s],
                                    in1=u0b[:, :, 2:5, 0:64], op=ALU.mult)
                    v.tensor_tensor(out=t2[:], in0=q8[:, :, 3:6, s],
                                    in1=u0b[:, :, 1:4, 0:64], op=ALU.mult)
                    v.tensor_tensor(out=uv5[:, :, 0:3, :], in0=t1[:], in1=t2[:],
                                    op=ALU.subtract)
                    v.tensor_copy(out=uv5[:, :, 3:5, :], in_=uv5[:, :, 0:2, :])
                    # kk = qv x uv  (into t1)
                    v.tensor_tensor(out=t2[:], in0=q8[:, :, 2:5, s],
                                    in1=uv5[:, :, 2:5, :], op=ALU.mult)
                    v.tensor_tensor(out=t1[:], in0=q8[:, :, 3:6, s],
                                    in1=uv5[:, :, 1:4, :], op=ALU.mult)
                    v.tensor_tensor(out=t1[:], in0=t2[:], in1=t1[:],
                                    op=ALU.subtract)
                    # mm = w*uv + kk ; bu = 2*mm + u0
                    wb = q8[:, :, 0, s].unsqueeze(2).to_broadcast([P, W, 3, 64])
                    v.tensor_tensor(out=t2[:], in0=wb, in1=uv5[:, :, 0:3, :],
                                    op=ALU.mult)
                    v.tensor_tensor(out=t1[:], in0=t2[:], in1=t1[:], op=ALU.add)
                    v.tensor_scalar_mul(t1[:], t1[:], 2.0)
                    v.tensor_tensor(out=bu[:, :, 0:3, s], in0=t1[:],
                                    in1=u0b[:, :, 0:3, 0:64], op=ALU.add)
                v.tensor_copy(out=bu[:, :, 3:5, :], in_=bu[:, :, 0:2, :])

        # ---------- post: b_v, m1, m2, staging ---------------------------
        with tc.tile_pool(name="pp", bufs=1) as pp:
            bv = pp.tile([P, W, 3, E], F16)
            bt = pp.tile([P, W, 3, E], F16, tag="bt")
            bs = pp.tile([P, W, 3, E], F16, tag="bs")
            bm = pp.tile([P, W, E], F16, tag="bm")
            # bv = normalize(cross(edges, bu))
            v.tensor_tensor(out=bt[:], in0=e5[:, :, 1:4, 1 : E + 1],
                            in1=bu[:, :, 2:5, :], op=ALU.mult)
            v.tensor_tensor(out=bs[:], in0=e5[:, :, 2:5, 1 : E + 1],
                            in1=bu[:, :, 1:4, :], op=ALU.mult)
            v.tensor_tensor(out=bv[:], in0=bt[:], in1=bs[:], op=ALU.subtract)
            v.tensor_tensor(out=bt[:], in0=bv[:], in1=bv[:], op=ALU.mult)
            v.tensor_tensor(out=bm[:], in0=bt[:, :, 0, :], in1=bt[:, :, 1, :],
                            op=ALU.add)
            v.tensor_tensor(out=bm[:], in0=bm[:], in1=bt[:, :, 2, :], op=ALU.add)
            sc.activation(bm[:], bm[:], AF.Sqrt, bias=c0[:])
            v.reciprocal(out=bm[:], in_=bm[:])
            rbb = bm[:].unsqueeze(2).to_broadcast([P, W, 3, E])
            v.tensor_tensor(out=bv[:], in0=bv[:], in1=rbb, op=ALU.mult)
            cb = lambda lo, hi: (cosf[:, :, lo:hi].unsqueeze(2)
                                 .to_broadcast([P, W, 3, hi - lo]))
            sbx = lambda lo, hi: (sinf[:, :, lo:hi].unsqueeze(2)
                                  .to_broadcast([P, W, 3, hi - lo]))
            with tc.tile_pool(name="stp", bufs=2) as stp:
                for ci in range(4):
                    lo, hi = ci * 32, ci * 32 + 32
                    m = 32
                    stg = stp.tile([P, W, m, 15], F16, tag="stg", name="stg")
                    tb = lambda fld: stg[:, :, :, fld * 3 : fld * 3 + 3].rearrange(
                        "p w n c -> p w c n")
                    sc.activation(tb(0), bu[:, :, 0:3, lo:hi], AF.Copy)
                    sc.activation(tb(1), bv[:, :, :, lo:hi], AF.Copy)
                    if ci == 0:
                        sc.activation(stg[:, :, 1:m, 6:9].rearrange(
                            "p w n c -> p w c n"), kb[:, :, :, 1:32], AF.Copy)
                        v.memset(stg[:, :, 0, 6:9], 0.0)
                    else:
                        sc.activation(tb(2), kb[:, :, :, lo:hi], AF.Copy)
                    # m1 = cos*bu + sin*bv ; m2 = cos*bv - sin*bu
                    v.tensor_tensor(out=bt[:, :, :, lo:hi], in0=cb(lo, hi),
                                    in1=bu[:, :, 0:3, lo:hi], op=ALU.mult)
                    v.tensor_tensor(out=bs[:, :, :, lo:hi], in0=sbx(lo, hi),
                                    in1=bv[:, :, :, lo:hi], op=ALU.mult)
                    v.tensor_tensor(out=tb(3), in0=bt[:, :, :, lo:hi],
                                    in1=bs[:, :, :, lo:hi], op=ALU.add)
                    v.tensor_tensor(out=bt[:, :, :, lo:hi], in0=cb(lo, hi),
                                    in1=bv[:, :, :, lo:hi], op=ALU.mult)
                    v.tensor_tensor(out=bs[:, :, :, lo:hi], in0=sbx(lo, hi),
                                    in1=bu[:, :, 0:3, lo:hi], op=ALU.mult)
                    v.tensor_tensor(out=tb(4), in0=bt[:, :, :, lo:hi],
                                    in1=bs[:, :, :, lo:hi], op=ALU.subtract)
                    nc.gpsimd.dma_start(outr[:, :, lo:hi, :, :], stg[:])

    return nc


def _split_excess_waits(nc):
    """Walrus encodes at most 1 sync wait per instruction; move excess waits
    onto NoOp carriers inserted just before, same engine."""
    MAXW = 1
    for func in nc.m.functions:
        for bb in func.blocks:
            insts = bb.instructions
            new_list = []
            changed = False
            for inst in insts:
                si = inst.sync_info
                waits = list(si.on_wait) if si is not None and si.on_wait else []
                if len(waits) > MAXW:
                    excess = waits[:-MAXW]
                    for j in range(0, len(excess), MAXW):
                        nop = mybir.InstNoOp(name=f"waitfix-{nc.next_id()}",
                                             engine=inst.engine)
                        nop.sync_info = mybir.SyncInfo(
                            on_wait=excess[j : j + MAXW], on_update=[])
                        new_list.append(nop)
                    si.on_wait = waits[-MAXW:]
                    changed = True
                new_list.append(inst)
            if changed:
                try:
                    bb.instructions = new_list
                except Exception:
                    insts.clear()
                    insts.extend(new_list)


def kernel(**inputs):
    verts = np.ascontiguousarray(inputs["verts"], dtype=np.float32)
    init_d = np.ascontiguousarray(inputs["init_direct"], dtype=np.float32)
    m_theta = np.ascontiguousarray(inputs["m_theta"], dtype=np.float32)
    restL = np.ascontiguousarray(inputs["restEdgeL"], dtype=np.float32)
    B = verts.shape[0]
    R = B // NCORES
    if "nc" not in _CACHE or _CACHE.get("R") != R:
        nc_new = build_nc(R)
        _split_excess_waits(nc_new)
        _CACHE["nc"] = nc_new
        _CACHE["R"] = R
    nc = _CACHE["nc"]
    in_maps = []
    for i in range(NCORES):
        sl = slice(i * R, (i + 1) * R)
        in_maps.append({
            "verts": verts[sl],
            "init_direct": init_d[sl],
            "m_theta": m_theta[sl],
            "restEdgeL": restL[sl],
        })
    try:
        from concourse.bass_utils import axon_active
        use_axon = axon_active()
    except Exception:
        use_axon = False
    if use_axon:
        # shard_map returns the full concatenated output; avoid an extra
        # 189MB split+concat round trip on the host.
        from concourse import bass2jax
        res = bass2jax.run_bass_via_pjrt(nc, in_maps, n_cores=NCORES)
        outs = [res[i]["out"] for i in range(NCORES)]
        def _root(a):
            while a.base is not None:
                a = a.base
            return a
        roots = [_root(o) for o in outs]
        base = roots[0]
        if (all(r is base for r in roots) and base.shape == (B, E, 5, 3)
                and base.flags.c_contiguous and base.dtype == np.float32):
            return base
        return np.concatenate(outs, axis=0)
    res = run_bass_kernel_spmd(nc, in_maps, core_ids=list(range(NCORES)))
    return np.concatenate([res.results[i]["out"] for i in range(NCORES)], axis=0)
